# revision 2
# baseline (speedup 1.0000x reference)
"""Trainium2 Bass kernel for nn_MultiHeadAttention_66322884984909 (v3).

Math (faithful to reference):
  Q = X @ W_Q.T reshaped (B, H, L, hd) via DIRECT reshape -> head h owns rows
  128h:128(h+1) of the projected (L, D) matrix, reinterpreted as (2048, 64).
  32 independent (batch, head) pairs; 8 cores x 4 pairs (2 groups of 2).

Structure:
  - Q/K projections computed PRE-TRANSPOSED: psum [128 (64h+j), 512 (p,l')]
    = W_blk.T @ X.T with all 4 pairs packed in the moving operand; strided
    PSUM->SBUF copies (split DVE/ACT) place qht/kht [64i+j, 16l'+t] directly.
    No DRAM bounce, no PE transposes.
  - V projection natural; key-block shuffle runs as per-(bb, group)
    SBUF->SBUF DMAs on the vector queue, ones column rides along.
  - Causal diagonal at 128-query granularity: partial-width S/PV matmuls,
    exp skips fully-masked subblocks, affine_select only on the four
    exact-diagonal 128x128 blocks (keep iff query >= key).
  - W_O loaded twice from HBM (both partition halves) for row-tiled Y.
  - Queues: sync = pure HBM loads; vector = dependent DMAs (shuffles, out).

No max-subtraction in softmax: logits ~ N(0, 64), exp stays finite in fp32.
Row sums ride as a 65th ones-column through P@V. All matmuls fp32r.
"""

import numpy as np

import concourse.bass as bass
from concourse import bacc
import concourse.mybir as mybir
import concourse.tile as tile
from concourse.bass_utils import run_bass_kernel_spmd
from concourse.masks import make_identity

F32 = mybir.dt.float32
F32R = mybir.dt.float32r
BF16 = mybir.dt.bfloat16
EXP = mybir.ActivationFunctionType.Exp
MASKV = -300.0  # exp(S + MASKV) == 0.0 in fp32 for all realistic logits

B, L, D = 2, 2048, 1024
H, HD = 16, 64
NCORES = 8
PPC = 4  # pairs per core
NG = PPC // 2  # groups of 2 pairs


def build_nc(repeat=1):
    nc = bacc.Bacc(trn_type="TRN2", target_bir_lowering=False, debug=False)

    xt = nc.declare_dram_parameter("xt", [128, 8, 512], F32R, isOutput=False)
    wq = nc.declare_dram_parameter("wq", [1024, 1024], F32R, isOutput=False)  # W_Q.T
    wk = nc.declare_dram_parameter("wk", [1024, 1024], F32R, isOutput=False)
    wv = nc.declare_dram_parameter("wv", [1024, 1024], F32R, isOutput=False)
    wo = nc.declare_dram_parameter("wo", [64, 16 * 1024], BF16, isOutput=False)
    bias = nc.declare_dram_parameter("bias", [128, 1024], F32, isOutput=False)
    out = nc.declare_dram_parameter("out", [PPC, 128, 1024], F32, isOutput=True)

    with tile.TileContext(nc) as tc:
        with (
            tc.tile_pool(name="consts", bufs=1) as consts,
            tc.tile_pool(name="headt", bufs=1) as headt,
            tc.tile_pool(name="bigps", bufs=2, space="PSUM") as bigps,
            tc.tile_pool(name="smallps", bufs=4, space="PSUM") as smallps,
            tc.tile_pool(name="projp", bufs=1) as projp,
            tc.tile_pool(name="wp", bufs=4) as wp,
            tc.tile_pool(name="onp", bufs=2) as onp,
            tc.tile_pool(name="ptp", bufs=3) as ptp,
            tc.tile_pool(name="dptp", bufs=3) as dptp,
            tc.tile_pool(name="rp", bufs=2) as rp,
            tc.tile_pool(name="yp", bufs=2) as ypool,
        ):
          for _rep in range(repeat):
            # causal-mask matmul constants (bf16): mask_sb[k, r] = MASKV if k < r
            ident_f = consts.tile([128, 128], F32, tag="idf")
            make_identity(nc, ident_f)
            ident_bf = consts.tile([128, 128], BF16, tag="idbf")
            nc.vector.tensor_copy(ident_bf, ident_f)
            mask_sb = consts.tile([128, 128], BF16, tag="mask")
            nc.vector.memset(mask_sb, MASKV)
            nc.gpsimd.affine_select(
                out=mask_sb, in_=mask_sb,
                compare_op=mybir.AluOpType.is_ge, fill=0.0,
                base=-1, channel_multiplier=-1, pattern=[[1, 128]],
            )

            qht2 = [headt.tile([128, 2048], F32R, tag=f"qht{g}", name=f"qht{g}")
                    for g in range(NG)]
            kht2 = [headt.tile([128, 2048], F32R, tag=f"kht{g}", name=f"kht{g}")
                    for g in range(NG)]
            # key-block-shuffled V+ones: vhs[key, 1040 p + 65 bb + c]
            vhs = headt.tile([128, PPC * 1040], F32R, tag="vhs", name="vhs")
            wo_sb = consts.tile([128, 16 * 1024], BF16, tag="wo")
            bias_sb = consts.tile([128, 1024], F32, tag="bias")

            if True:
                xt_all = projp.tile([128, 8, 512], F32R, tag="xt", name="xt_all")

                def load_w(wparam, nm):
                    quarters = []
                    for c2 in range(4):
                        t = wp.tile([128, 2, 1024], F32R, tag="w",
                                    name=f"w{nm}{c2}")
                        nc.sync.dma_start(
                            out=t,
                            in_=wparam.rearrange("(c p) j -> p c j", p=128)[
                                :, 2 * c2:2 * c2 + 2, :])
                        quarters.append(t)
                    return quarters

                # --- Q/K phases: transposed projection ---
                # startup: interleave xt chunks with wq quarters so the first
                # matmul's inputs land first on the DMA engines
                wq_h = []
                for c2 in range(4):
                    nc.sync.dma_start(out=xt_all[:, 2 * c2, :],
                                      in_=xt[:, 2 * c2, :])
                    nc.sync.dma_start(out=xt_all[:, 2 * c2 + 1, :],
                                      in_=xt[:, 2 * c2 + 1, :])
                    t = wp.tile([128, 2, 1024], F32R, tag="w", name=f"wq{c2}")
                    nc.sync.dma_start(
                        out=t,
                        in_=wq.rearrange("(c p) j -> p c j", p=128)[
                            :, 2 * c2:2 * c2 + 2, :])
                    wq_h.append(t)
                for (phase_i, wparam, dst2) in ((0, wq, qht2), (1, wk, kht2)):
                    w_h = wq_h if phase_i == 0 else load_w(wparam, f"qk{phase_i}")
                    for jb in range(8):
                        half, j2, jj = jb >> 2, (jb >> 1) & 1, jb & 1
                        ps = smallps.tile([128, 512], F32, tag="small",
                                          name=f"qkps{phase_i}_{jb}")
                        for kc in range(8):
                            nc.tensor.matmul(
                                ps,
                                lhsT=w_h[kc >> 1][:, kc & 1,
                                                  128 * jb:128 * (jb + 1)],
                                rhs=xt_all[:, kc, :],
                                start=(kc == 0), stop=(kc == 7),
                            )
                        src_r = ps.rearrange("q (pp lp) -> q pp lp", pp=4)
                        for h in range(2):
                            for p in range(PPC):
                                g, i = p >> 1, p & 1
                                dst_r = dst2[g].rearrange(
                                    "q (lp half j2 jj hh) -> q half j2 jj hh lp",
                                    lp=128, half=2, j2=2, jj=2)
                                dst = dst_r[64 * i:64 * i + 64,
                                            half, j2, jj, h, :]
                                src = src_r[64 * h:64 * h + 64, p, :]
                                if (2 * h + p) % 2 == 0:
                                    nc.vector.tensor_copy(dst, src)
                                else:
                                    nc.scalar.copy(dst, src)

                # --- V phase: natural projection + SBUF->SBUF shuffle ---
                w_h = load_w(wv, "v")

                # W_O + bias loads: pure loads, issued after the critical wv
                # chunks so they don't stall the V phase on DMA bandwidth.
                nc.sync.dma_start(out=wo_sb[0:64, :], in_=wo[:])
                nc.sync.dma_start(out=wo_sb[64:128, :], in_=wo[:])
                nc.sync.dma_start(out=bias_sb, in_=bias[:])
                # vf t-major over all pairs: vf[l', 260 t + 65 p + c]
                vf = projp.tile([128, 16 * 260], F32R, tag="vf", name="vf")
                vf_r = vf.rearrange("q (t pp c) -> q t pp c", pp=PPC, c=65)
                # ones columns: fill everything with 1.0; the V copies below
                # overwrite the c<64 lanes, leaving 1.0 at c==64
                # (memset doesn't take f32r — bitcast the view to f32)
                nc.gpsimd.memset(vf[:].bitcast(F32), 1.0)
                for p in range(PPC):
                    ps = bigps.tile([128, 1024], F32, tag="big", name=f"vps{p}")
                    for jh in range(2):
                        for kc in range(8):
                            nc.tensor.matmul(
                                ps[:, 512 * jh:512 * (jh + 1)],
                                lhsT=xt_all[:, kc, 128 * p:128 * (p + 1)],
                                rhs=w_h[kc >> 1][:, kc & 1,
                                                 512 * jh:512 * (jh + 1)],
                                start=(kc == 0), stop=(kc == 7),
                            )
                    for jh in range(2):
                        nc.vector.tensor_copy(
                            vf_r[:, 8 * jh:8 * (jh + 1), p, 0:64],
                            ps[:, 512 * jh:512 * (jh + 1)].rearrange(
                                "q (t j) -> q t j", j=64))
                # shuffle: vhs[16dl+t, 1040 p + 65 bb + c] = vf[8bb+dl, 260t+65p+c]
                # bb ascending (attention consumes low bb first); spread the
                # 16 DMAs over three queues so descriptor gen runs in parallel
                squeues = [nc.sync, nc.scalar, nc.gpsimd]
                for bb in range(16):
                    src = vf[8 * bb:8 * bb + 8, :]
                    dst = vhs.rearrange("q (pp bc) -> q pp bc", pp=PPC)[
                        :, :, 65 * bb:65 * bb + 65]
                    squeues[bb % 3].dma_start(out=dst, in_=src)

            with (
                tc.tile_pool(name="onp", bufs=2) as onp,
                tc.tile_pool(name="ptp", bufs=3) as ptp,
                tc.tile_pool(name="dptp", bufs=3) as dptp,
                tc.tile_pool(name="rp", bufs=2) as rp,
                tc.tile_pool(name="yp", bufs=2) as ypool,
            ):
                for g in range(NG):
                    onorm2 = emit_attention(nc, tc, g, qht2, kht2, vhs,
                                            bigps, smallps, ptp, dptp, rp, onp,
                                            mask_sb, ident_bf)
                    emit_y(nc, g, onorm2, wo_sb, bias_sb, smallps, ypool, out)

    nc.finalize()
    return nc


def emit_attention(nc, tc, g, qht2, kht2, vhsg, bigps, smallps, ptp, dptp,
                   rp, onp, mask_sb, ident_bf):
    onorm2 = onp.tile([128, 2048], BF16, tag="onorm", name=f"onorm{g}")
    for a in range(4):
        pvf = [smallps.tile([128, 512], F32, tag="small", name=f"pv{g}_{a}_{i}")
               for i in range(2)]
        pvs = [t[0:65, :] for t in pvf]
        nmm = [0, 0]  # matmuls issued into pvs[i] (for start flag)

        # off-diagonal key blocks, in pairs
        for bbp in range(2 * a):
            for i in range(2):
                sts = bigps.tile([128, 1024], F32, tag="big",
                                 name=f"sts{g}_{a}_{bbp}_{i}")
                for q2 in range(2):
                    bb = 2 * bbp + q2
                    nc.tensor.matmul(
                        sts[:, 512 * q2:512 * (q2 + 1)],
                        lhsT=kht2[g][64 * i:64 * i + 64,
                                     128 * bb:128 * (bb + 1)],
                        rhs=qht2[g][64 * i:64 * i + 64,
                                    512 * a:512 * (a + 1)],
                        start=True, stop=True,
                    )
                pt = ptp.tile([128, 1024], F32R, tag="pt", name=f"pt{i}")
                nc.scalar.activation(pt, sts, EXP)
                for q2 in range(2):
                    bb = 2 * bbp + q2
                    nc.tensor.matmul(
                        pvs[i],
                        lhsT=vhsg[:, 1040 * (2 * g + i) + 65 * bb:
                                 1040 * (2 * g + i) + 65 * bb + 65],
                        rhs=pt[:, 512 * q2:512 * (q2 + 1)],
                        start=(nmm[i] == 0), stop=False,
                    )
                    nmm[i] += 1

        # diagonal: 4 key blocks, partial widths (skip fully-masked area)
        # psum layout: dst1 = [s0: 0:512 | s1: 512:896 | s3: 896:1024],
        #              dst2 = [s2: 0:256]
        # The exact-diagonal 128x128 of each block gets MASKV added via a
        # bf16 mask-matmul (accumulated into psum) so exp() yields 0 there —
        # no gpsimd affine_select on the critical path.
        DOFF = {0: 0, 1: 512, 3: 896, 2: 1024}
        dst1 = [None, None]
        dst2 = [None, None]
        for i in range(2):
            dst1[i] = bigps.tile([128, 1024], F32, tag="big", name=f"dg1_{i}")
            dst2[i] = smallps.tile([128, 512], F32, tag="small", name=f"dg2_{i}")
            # one accumulation group per psum bank:
            #   bank0 of dst1: s0 + mask0;  bank1 of dst1: s1, s3, mask1, mask3
            #   dst2: s2 + mask2.  s3 start=False overwrites its untouched
            #   region (has_written unset there), keeping bank1 a single group.
            for s in range(4):
                w = 512 - 128 * s
                tgt = dst1[i][:, DOFF[s]:DOFF[s] + w] if s != 2 \
                    else dst2[i][:, 0:256]
                nc.tensor.matmul(
                    tgt,
                    lhsT=kht2[g][64 * i:64 * i + 64,
                                 128 * (4 * a + s):128 * (4 * a + s + 1)],
                    rhs=qht2[g][64 * i:64 * i + 64,
                                512 * a + 128 * s:512 * (a + 1)],
                    start=(s != 3), stop=False,
                )
        for i in range(2):
            for s, last in ((0, True), (1, False), (3, True), (2, True)):
                tgt = dst1[i][:, DOFF[s]:DOFF[s] + 128] if s != 2 \
                    else dst2[i][:, 0:128]
                nc.tensor.matmul(
                    tgt, lhsT=mask_sb, rhs=ident_bf,
                    start=False, stop=last,
                    skip_group_check=True,
                )
        for i in range(2):
            dpt = dptp.tile([128, 1280], F32R, tag="dpt", name=f"dpt{i}")
            nc.scalar.activation(dpt[:, 0:512], dst1[i][:, 0:512], EXP)
            nc.scalar.activation(dpt[:, 512:1024], dst1[i][:, 512:1024], EXP)
            nc.scalar.activation(dpt[:, 1024:1280], dst2[i][:, 0:256], EXP)
            for s in range(4):
                w = 512 - 128 * s
                bb = 4 * a + s
                nc.tensor.matmul(
                    pvs[i][:, 128 * s:512],
                    lhsT=vhsg[:, 1040 * (2 * g + i) + 65 * bb:
                                 1040 * (2 * g + i) + 65 * bb + 65],
                    rhs=dpt[:, DOFF[s]:DOFF[s] + w],
                    start=(nmm[i] == 0), stop=(s == 3),
                )
                nmm[i] += 1

        for i in range(2):
            # evacuate psum promptly so the pool recycles without waiting on
            # the normalization chain
            pvsb = rp.tile([65, 512], F32, tag="pvsb", name=f"pvsb{i}")
            nc.vector.tensor_copy(pvsb, pvs[i])
            r1 = rp.tile([1, 512], F32, tag="r1", name="r1_t")
            nc.vector.reciprocal(r1, pvsb[64:65, :])
            rb = rp.tile([64, 512], F32, tag="rb", name="rb_t")
            nc.gpsimd.partition_broadcast(rb, r1)
            nc.vector.tensor_mul(
                onorm2[64 * i:64 * i + 64, 512 * a:512 * (a + 1)],
                pvsb[0:64, :], rb)

    return onorm2


def emit_y(nc, g, onorm2, wo_sb, bias_sb, smallps, ypool, out):
    onorm_r = onorm2.rearrange("q (i t) -> q t i", t=16)
    ysbs = [ypool.tile([128, 1024], F32, tag="ysb", name=f"ysb{g}_{i}")
            for i in range(2)]
    for jh in range(2):
        yps = [smallps.tile([128, 512], F32, tag="small", name=f"yps{g}_{i}")
               for i in range(2)]
        for t in range(16):
            for i in range(2):
                nc.tensor.matmul(
                    yps[i],
                    lhsT=onorm_r[64 * i:64 * i + 64, t, :],
                    rhs=wo_sb[64 * i:64 * i + 64,
                              t * 1024 + jh * 512:t * 1024 + (jh + 1) * 512],
                    start=(t == 0), stop=(t == 15),
                )
        for i in range(2):
            nc.vector.tensor_add(
                ysbs[i][:, jh * 512:(jh + 1) * 512], yps[i],
                bias_sb[:, jh * 512:(jh + 1) * 512])
            nc.sync.dma_start(
                out=out[2 * g + i][:, jh * 512:(jh + 1) * 512],
                in_=ysbs[i][:, jh * 512:(jh + 1) * 512])



def _host_prep(input_seq_embs, W_Q, W_K, W_V, W_O, b_O):
    X = np.asarray(input_seq_embs, dtype=np.float32)
    WQ = np.asarray(W_Q, dtype=np.float32)
    WK = np.asarray(W_K, dtype=np.float32)
    WV = np.asarray(W_V, dtype=np.float32)
    WO = np.asarray(W_O, dtype=np.float32)
    bO = np.asarray(b_O, dtype=np.float32)

    wq_arr = np.ascontiguousarray(WQ.T)
    wk_arr = np.ascontiguousarray(WK.T)
    wv_arr = np.ascontiguousarray(WV.T)
    # wo[j2, 1024 t + jo] = W_O.T[64 t + j2, jo]
    import ml_dtypes
    wo_arr = np.ascontiguousarray(
        WO.T.reshape(16, 64, 1024).transpose(1, 0, 2).reshape(64, 16 * 1024)
    ).astype(ml_dtypes.bfloat16)
    bias_arr = np.ascontiguousarray(
        np.broadcast_to(bO, (128, 1024)).astype(np.float32))

    in_maps = []
    for c in range(NCORES):
        # xt[k % 128, k // 128, 128 p + l'] = X_pair_p[l', k]
        xt_all = np.empty((128, 8, 512), dtype=np.float32)
        for p in range(PPC):
            gidx = PPC * c + p
            bb, hh = gidx // H, gidx % H
            xs_t = X[bb, 128 * hh:128 * (hh + 1), :].T  # (1024, 128)
            xt_all[:, :, 128 * p:128 * (p + 1)] = (
                xs_t.reshape(8, 128, 128).transpose(1, 0, 2))
        in_maps.append({
            "xt": np.ascontiguousarray(xt_all),
            "wq": wq_arr, "wk": wk_arr, "wv": wv_arr, "wo": wo_arr,
            "bias": bias_arr,
        })
    return in_maps


_CACHED_NC = None


def get_nc():
    global _CACHED_NC
    if _CACHED_NC is None:
        _CACHED_NC = build_nc()
    return _CACHED_NC


def kernel(**inputs) -> np.ndarray:
    nc = get_nc()
    in_maps = _host_prep(**inputs)
    res = run_bass_kernel_spmd(nc, in_maps, list(range(NCORES)))
    out = np.empty((B, L, D), dtype=np.float32)
    for c in range(NCORES):
        y = res.results[c]["out"]  # (4, 128, 1024)
        for p in range(PPC):
            gidx = PPC * c + p
            bb, hh = gidx // H, gidx % H
            out[bb, 128 * hh:128 * (hh + 1), :] = y[p]
    return out


# revision 3
# speedup vs baseline: 5.6709x; 5.6709x over previous
"""Trainium2 Bass kernel for nn_MultiHeadAttention_66322884984909.

Math (faithful to reference):
  Q = X @ W_Q.T reshaped (B, H, L, hd) via DIRECT reshape -> head h owns rows
  128h:128(h+1) of the projected (L, D) matrix, reinterpreted as (2048, 64).
  32 independent (batch, head) pairs; 8 cores x 4 pairs (2 groups of 2).

Structure:
  - Q/K projections computed PRE-TRANSPOSED: psum [128 (64h+j), 512 (p,l')]
    = W_blk.T @ X.T with all 4 pairs packed in the moving operand; strided
    PSUM->SBUF copies (alternating DVE/ACT) place qht/kht [64i+j, 16l'+t]
    directly. No DRAM bounce, no PE transposes.
  - V projection natural (t-major layout); the key-block shuffle
    vhs[16dl+t, 65bb+c] = vf[8bb+dl, ...] runs as 16 SBUF->SBUF DMAs
    spread over the sync/scalar/gpsimd queues; ones column rides along.
  - Causal diagonal at 128-query granularity: partial-width S/PV matmuls
    skip the fully-masked area; the exact-diagonal 128x128 of each key
    block gets -300 added via a bf16 mask-matmul accumulated into psum, so
    exp() yields exact 0 there with no gpsimd affine_select on the
    critical path.
  - S and Y matmuls have K=64 and are row-tiled (64x128 array tiles via
    base partitions) so the i=0/i=1 pair runs concurrently.
  - W_O and the normalized attention output are bf16 (errs ~0.2%); W_O is
    loaded twice from HBM (both partition halves) for the row-tiled Y.
  - Startup: xt chunks interleaved with wq quarters on the sync queue so
    the first matmul's operands land first.
  - All pools live at TileContext scope so repeated bodies (slope timing)
    pipeline across repetitions.

No max-subtraction in softmax: logits ~ N(0, 64), exp stays finite in fp32.
Row sums ride as a 65th ones-column through P@V. Matmuls in fp32r except
the bf16 mask/Y path.
"""

import numpy as np

import concourse.bass as bass
from concourse import bacc
import concourse.mybir as mybir
import concourse.tile as tile
from concourse.bass_utils import run_bass_kernel_spmd
from concourse.masks import make_identity

F32 = mybir.dt.float32
F32R = mybir.dt.float32r
BF16 = mybir.dt.bfloat16
EXP = mybir.ActivationFunctionType.Exp
MASKV = -300.0  # exp(S + MASKV) == 0.0 in fp32 for all realistic logits

B, L, D = 2, 2048, 1024
H, HD = 16, 64
NCORES = 8
PPC = 4  # pairs per core
NG = PPC // 2  # groups of 2 pairs


def build_nc(repeat=1):
    nc = bacc.Bacc(trn_type="TRN2", target_bir_lowering=False, debug=False)

    xt = nc.declare_dram_parameter("xt", [128, 8, 512], F32R, isOutput=False)
    wq = nc.declare_dram_parameter("wq", [1024, 1024], F32R, isOutput=False)  # W_Q.T
    wk = nc.declare_dram_parameter("wk", [1024, 1024], F32R, isOutput=False)
    wv = nc.declare_dram_parameter("wv", [1024, 1024], F32R, isOutput=False)
    wo = nc.declare_dram_parameter("wo", [64, 16 * 1024], BF16, isOutput=False)
    bias = nc.declare_dram_parameter("bias", [128, 1024], F32, isOutput=False)
    out = nc.declare_dram_parameter("out", [PPC, 128, 1024], F32, isOutput=True)

    with tile.TileContext(nc) as tc:
        with (
            tc.tile_pool(name="consts", bufs=1) as consts,
            tc.tile_pool(name="headt", bufs=1) as headt,
            tc.tile_pool(name="bigps", bufs=2, space="PSUM") as bigps,
            tc.tile_pool(name="smallps", bufs=4, space="PSUM") as smallps,
            tc.tile_pool(name="projp", bufs=1) as projp,
            tc.tile_pool(name="wp", bufs=4) as wp,
            tc.tile_pool(name="onp", bufs=2) as onp,
            tc.tile_pool(name="ptp", bufs=3) as ptp,
            tc.tile_pool(name="dptp", bufs=3) as dptp,
            tc.tile_pool(name="rp", bufs=2) as rp,
            tc.tile_pool(name="yp", bufs=2) as ypool,
        ):
          for _rep in range(repeat):
            # causal-mask matmul constants (bf16): mask_sb[k, r] = MASKV if k < r
            ident_f = consts.tile([128, 128], F32, tag="idf")
            make_identity(nc, ident_f)
            ident_bf = consts.tile([128, 128], BF16, tag="idbf")
            nc.vector.tensor_copy(ident_bf, ident_f)
            mask_sb = consts.tile([128, 128], BF16, tag="mask")
            nc.vector.memset(mask_sb, MASKV)
            nc.gpsimd.affine_select(
                out=mask_sb, in_=mask_sb,
                compare_op=mybir.AluOpType.is_ge, fill=0.0,
                base=-1, channel_multiplier=-1, pattern=[[1, 128]],
            )

            qht2 = [headt.tile([128, 2048], F32R, tag=f"qht{g}", name=f"qht{g}")
                    for g in range(NG)]
            kht2 = [headt.tile([128, 2048], F32R, tag=f"kht{g}", name=f"kht{g}")
                    for g in range(NG)]
            # key-block-shuffled V+ones: vhs[key, 1040 p + 65 bb + c]
            vhs = headt.tile([128, PPC * 1040], F32R, tag="vhs", name="vhs")
            wo_sb = consts.tile([128, 16 * 1024], BF16, tag="wo")
            bias_sb = consts.tile([128, 1024], F32, tag="bias")

            if True:
                xt_all = projp.tile([128, 8, 512], F32R, tag="xt", name="xt_all")

                def load_w(wparam, nm):
                    quarters = []
                    for c2 in range(4):
                        t = wp.tile([128, 2, 1024], F32R, tag="w",
                                    name=f"w{nm}{c2}")
                        nc.sync.dma_start(
                            out=t,
                            in_=wparam.rearrange("(c p) j -> p c j", p=128)[
                                :, 2 * c2:2 * c2 + 2, :])
                        quarters.append(t)
                    return quarters

                # --- Q/K phases: transposed projection ---
                # startup: interleave xt chunks with wq quarters so the first
                # matmul's inputs land first on the DMA engines
                wq_h = []
                for c2 in range(4):
                    nc.sync.dma_start(out=xt_all[:, 2 * c2, :],
                                      in_=xt[:, 2 * c2, :])
                    nc.sync.dma_start(out=xt_all[:, 2 * c2 + 1, :],
                                      in_=xt[:, 2 * c2 + 1, :])
                    t = wp.tile([128, 2, 1024], F32R, tag="w", name=f"wq{c2}")
                    nc.sync.dma_start(
                        out=t,
                        in_=wq.rearrange("(c p) j -> p c j", p=128)[
                            :, 2 * c2:2 * c2 + 2, :])
                    wq_h.append(t)
                for (phase_i, wparam, dst2) in ((0, wq, qht2), (1, wk, kht2)):
                    w_h = wq_h if phase_i == 0 else load_w(wparam, f"qk{phase_i}")
                    for jb in range(8):
                        half, j2, jj = jb >> 2, (jb >> 1) & 1, jb & 1
                        ps = smallps.tile([128, 512], F32, tag="small",
                                          name=f"qkps{phase_i}_{jb}")
                        for kc in range(8):
                            nc.tensor.matmul(
                                ps,
                                lhsT=w_h[kc >> 1][:, kc & 1,
                                                  128 * jb:128 * (jb + 1)],
                                rhs=xt_all[:, kc, :],
                                start=(kc == 0), stop=(kc == 7),
                            )
                        src_r = ps.rearrange("q (pp lp) -> q pp lp", pp=4)
                        for h in range(2):
                            for p in range(PPC):
                                g, i = p >> 1, p & 1
                                dst_r = dst2[g].rearrange(
                                    "q (lp half j2 jj hh) -> q half j2 jj hh lp",
                                    lp=128, half=2, j2=2, jj=2)
                                dst = dst_r[64 * i:64 * i + 64,
                                            half, j2, jj, h, :]
                                src = src_r[64 * h:64 * h + 64, p, :]
                                if (2 * h + p) % 2 == 0:
                                    nc.vector.tensor_copy(dst, src)
                                else:
                                    nc.scalar.copy(dst, src)

                # --- V phase: natural projection + SBUF->SBUF shuffle ---
                w_h = load_w(wv, "v")

                # W_O + bias loads: pure loads, issued after the critical wv
                # chunks so they don't stall the V phase on DMA bandwidth.
                nc.sync.dma_start(out=wo_sb[0:64, :], in_=wo[:])
                nc.sync.dma_start(out=wo_sb[64:128, :], in_=wo[:])
                nc.sync.dma_start(out=bias_sb, in_=bias[:])
                # vf t-major over all pairs: vf[l', 260 t + 65 p + c]
                vf = projp.tile([128, 16 * 260], F32R, tag="vf", name="vf")
                vf_r = vf.rearrange("q (t pp c) -> q t pp c", pp=PPC, c=65)
                # ones columns: fill everything with 1.0; the V copies below
                # overwrite the c<64 lanes, leaving 1.0 at c==64
                # (memset doesn't take f32r — bitcast the view to f32)
                nc.gpsimd.memset(vf[:].bitcast(F32), 1.0)
                for p in range(PPC):
                    ps = bigps.tile([128, 1024], F32, tag="big", name=f"vps{p}")
                    for jh in range(2):
                        for kc in range(8):
                            nc.tensor.matmul(
                                ps[:, 512 * jh:512 * (jh + 1)],
                                lhsT=xt_all[:, kc, 128 * p:128 * (p + 1)],
                                rhs=w_h[kc >> 1][:, kc & 1,
                                                 512 * jh:512 * (jh + 1)],
                                start=(kc == 0), stop=(kc == 7),
                            )
                    for jh in range(2):
                        nc.vector.tensor_copy(
                            vf_r[:, 8 * jh:8 * (jh + 1), p, 0:64],
                            ps[:, 512 * jh:512 * (jh + 1)].rearrange(
                                "q (t j) -> q t j", j=64))
                # shuffle: vhs[16dl+t, 1040 p + 65 bb + c] = vf[8bb+dl, 260t+65p+c]
                # bb ascending (attention consumes low bb first); spread the
                # 16 DMAs over three queues so descriptor gen runs in parallel
                squeues = [nc.sync, nc.scalar, nc.gpsimd]
                for bb in range(16):
                    src = vf[8 * bb:8 * bb + 8, :]
                    dst = vhs.rearrange("q (pp bc) -> q pp bc", pp=PPC)[
                        :, :, 65 * bb:65 * bb + 65]
                    squeues[bb % 3].dma_start(out=dst, in_=src)

            with (
                tc.tile_pool(name="onp", bufs=2) as onp,
                tc.tile_pool(name="ptp", bufs=3) as ptp,
                tc.tile_pool(name="dptp", bufs=3) as dptp,
                tc.tile_pool(name="rp", bufs=2) as rp,
                tc.tile_pool(name="yp", bufs=2) as ypool,
            ):
                for g in range(NG):
                    onorm2 = emit_attention(nc, tc, g, qht2, kht2, vhs,
                                            bigps, smallps, ptp, dptp, rp, onp,
                                            mask_sb, ident_bf)
                    emit_y(nc, g, onorm2, wo_sb, bias_sb, smallps, ypool, out)

    nc.finalize()
    return nc


def emit_attention(nc, tc, g, qht2, kht2, vhsg, bigps, smallps, ptp, dptp,
                   rp, onp, mask_sb, ident_bf):
    onorm2 = onp.tile([128, 2048], BF16, tag="onorm", name=f"onorm{g}")
    for a in range(4):
        pvf = [smallps.tile([128, 512], F32, tag="small", name=f"pv{g}_{a}_{i}")
               for i in range(2)]
        pvs = [t[0:65, :] for t in pvf]
        nmm = [0, 0]  # matmuls issued into pvs[i] (for start flag)

        # off-diagonal key blocks, in pairs
        for bbp in range(2 * a):
            for i in range(2):
                sts = bigps.tile([128, 1024], F32, tag="big",
                                 name=f"sts{g}_{a}_{bbp}_{i}")
                for q2 in range(2):
                    bb = 2 * bbp + q2
                    nc.tensor.matmul(
                        sts[:, 512 * q2:512 * (q2 + 1)],
                        lhsT=kht2[g][64 * i:64 * i + 64,
                                     128 * bb:128 * (bb + 1)],
                        rhs=qht2[g][64 * i:64 * i + 64,
                                    512 * a:512 * (a + 1)],
                        start=True, stop=True,
                    )
                pt = ptp.tile([128, 1024], F32R, tag="pt", name=f"pt{i}")
                nc.scalar.activation(pt, sts, EXP)
                for q2 in range(2):
                    bb = 2 * bbp + q2
                    nc.tensor.matmul(
                        pvs[i],
                        lhsT=vhsg[:, 1040 * (2 * g + i) + 65 * bb:
                                 1040 * (2 * g + i) + 65 * bb + 65],
                        rhs=pt[:, 512 * q2:512 * (q2 + 1)],
                        start=(nmm[i] == 0), stop=False,
                    )
                    nmm[i] += 1

        # diagonal: 4 key blocks, partial widths (skip fully-masked area)
        # psum layout: dst1 = [s0: 0:512 | s1: 512:896 | s3: 896:1024],
        #              dst2 = [s2: 0:256]
        # The exact-diagonal 128x128 of each block gets MASKV added via a
        # bf16 mask-matmul (accumulated into psum) so exp() yields 0 there —
        # no gpsimd affine_select on the critical path.
        DOFF = {0: 0, 1: 512, 3: 896, 2: 1024}
        dst1 = [None, None]
        dst2 = [None, None]
        for i in range(2):
            dst1[i] = bigps.tile([128, 1024], F32, tag="big", name=f"dg1_{i}")
            dst2[i] = smallps.tile([128, 512], F32, tag="small", name=f"dg2_{i}")
            # one accumulation group per psum bank:
            #   bank0 of dst1: s0 + mask0;  bank1 of dst1: s1, s3, mask1, mask3
            #   dst2: s2 + mask2.  s3 start=False overwrites its untouched
            #   region (has_written unset there), keeping bank1 a single group.
            for s in range(4):
                w = 512 - 128 * s
                tgt = dst1[i][:, DOFF[s]:DOFF[s] + w] if s != 2 \
                    else dst2[i][:, 0:256]
                nc.tensor.matmul(
                    tgt,
                    lhsT=kht2[g][64 * i:64 * i + 64,
                                 128 * (4 * a + s):128 * (4 * a + s + 1)],
                    rhs=qht2[g][64 * i:64 * i + 64,
                                512 * a + 128 * s:512 * (a + 1)],
                    start=(s != 3), stop=False,
                )
        for i in range(2):
            for s, last in ((0, True), (1, False), (3, True), (2, True)):
                tgt = dst1[i][:, DOFF[s]:DOFF[s] + 128] if s != 2 \
                    else dst2[i][:, 0:128]
                nc.tensor.matmul(
                    tgt, lhsT=mask_sb, rhs=ident_bf,
                    start=False, stop=last,
                    skip_group_check=True,
                )
        for i in range(2):
            dpt = dptp.tile([128, 1280], F32R, tag="dpt", name=f"dpt{i}")
            nc.scalar.activation(dpt[:, 0:512], dst1[i][:, 0:512], EXP)
            nc.scalar.activation(dpt[:, 512:1024], dst1[i][:, 512:1024], EXP)
            nc.scalar.activation(dpt[:, 1024:1280], dst2[i][:, 0:256], EXP)
            for s in range(4):
                w = 512 - 128 * s
                bb = 4 * a + s
                nc.tensor.matmul(
                    pvs[i][:, 128 * s:512],
                    lhsT=vhsg[:, 1040 * (2 * g + i) + 65 * bb:
                                 1040 * (2 * g + i) + 65 * bb + 65],
                    rhs=dpt[:, DOFF[s]:DOFF[s] + w],
                    start=(nmm[i] == 0), stop=(s == 3),
                )
                nmm[i] += 1

        for i in range(2):
            # evacuate psum promptly so the pool recycles without waiting on
            # the normalization chain
            pvsb = rp.tile([65, 512], F32, tag="pvsb", name=f"pvsb{i}")
            nc.vector.tensor_copy(pvsb, pvs[i])
            r1 = rp.tile([1, 512], F32, tag="r1", name="r1_t")
            nc.vector.reciprocal(r1, pvsb[64:65, :])
            rb = rp.tile([64, 512], F32, tag="rb", name="rb_t")
            nc.gpsimd.partition_broadcast(rb, r1)
            nc.vector.tensor_mul(
                onorm2[64 * i:64 * i + 64, 512 * a:512 * (a + 1)],
                pvsb[0:64, :], rb)

    return onorm2


def emit_y(nc, g, onorm2, wo_sb, bias_sb, smallps, ypool, out):
    onorm_r = onorm2.rearrange("q (i t) -> q t i", t=16)
    ysbs = [ypool.tile([128, 1024], F32, tag="ysb", name=f"ysb{g}_{i}")
            for i in range(2)]
    for jh in range(2):
        yps = [smallps.tile([128, 512], F32, tag="small", name=f"yps{g}_{i}")
               for i in range(2)]
        for t in range(16):
            for i in range(2):
                nc.tensor.matmul(
                    yps[i],
                    lhsT=onorm_r[64 * i:64 * i + 64, t, :],
                    rhs=wo_sb[64 * i:64 * i + 64,
                              t * 1024 + jh * 512:t * 1024 + (jh + 1) * 512],
                    start=(t == 0), stop=(t == 15),
                )
        for i in range(2):
            nc.vector.tensor_add(
                ysbs[i][:, jh * 512:(jh + 1) * 512], yps[i],
                bias_sb[:, jh * 512:(jh + 1) * 512])
            nc.sync.dma_start(
                out=out[2 * g + i][:, jh * 512:(jh + 1) * 512],
                in_=ysbs[i][:, jh * 512:(jh + 1) * 512])



def _host_prep(input_seq_embs, W_Q, W_K, W_V, W_O, b_O):
    X = np.asarray(input_seq_embs, dtype=np.float32)
    WQ = np.asarray(W_Q, dtype=np.float32)
    WK = np.asarray(W_K, dtype=np.float32)
    WV = np.asarray(W_V, dtype=np.float32)
    WO = np.asarray(W_O, dtype=np.float32)
    bO = np.asarray(b_O, dtype=np.float32)

    wq_arr = np.ascontiguousarray(WQ.T)
    wk_arr = np.ascontiguousarray(WK.T)
    wv_arr = np.ascontiguousarray(WV.T)
    # wo[j2, 1024 t + jo] = W_O.T[64 t + j2, jo]
    import ml_dtypes
    wo_arr = np.ascontiguousarray(
        WO.T.reshape(16, 64, 1024).transpose(1, 0, 2).reshape(64, 16 * 1024)
    ).astype(ml_dtypes.bfloat16)
    bias_arr = np.ascontiguousarray(
        np.broadcast_to(bO, (128, 1024)).astype(np.float32))

    in_maps = []
    for c in range(NCORES):
        # xt[k % 128, k // 128, 128 p + l'] = X_pair_p[l', k]
        xt_all = np.empty((128, 8, 512), dtype=np.float32)
        for p in range(PPC):
            gidx = PPC * c + p
            bb, hh = gidx // H, gidx % H
            xs_t = X[bb, 128 * hh:128 * (hh + 1), :].T  # (1024, 128)
            xt_all[:, :, 128 * p:128 * (p + 1)] = (
                xs_t.reshape(8, 128, 128).transpose(1, 0, 2))
        in_maps.append({
            "xt": np.ascontiguousarray(xt_all),
            "wq": wq_arr, "wk": wk_arr, "wv": wv_arr, "wo": wo_arr,
            "bias": bias_arr,
        })
    return in_maps


_CACHED_NC = None


def get_nc():
    global _CACHED_NC
    if _CACHED_NC is None:
        _CACHED_NC = build_nc()
    return _CACHED_NC


def kernel(**inputs) -> np.ndarray:
    nc = get_nc()
    in_maps = _host_prep(**inputs)
    res = run_bass_kernel_spmd(nc, in_maps, list(range(NCORES)))
    out = np.empty((B, L, D), dtype=np.float32)
    for c in range(NCORES):
        y = res.results[c]["out"]  # (4, 128, 1024)
        for p in range(PPC):
            gidx = PPC * c + p
            bb, hh = gidx // H, gidx % H
            out[bb, 128 * hh:128 * (hh + 1), :] = y[p]
    return out
